# revision 22
# baseline (speedup 1.0000x reference)
"""BoxFilter kernel for Trainium2 (8 NeuronCores).

Computes out[b,0,i,j] = sum_{c} sum_{|di|<=15} sum_{|dj|<=15} x[b,c,i+di,j+dj]
(edge-clamped 31x31 box filter over the channel-summed image), matching the
reference cumsum + shifted-diff formulation exactly (separable box sums).

Sharding: data-parallel over (batch, H-half) -> 8 shards, no cross-core
communication. Each core receives a host-padded [3, 1056, 2048] slab
(16 halo rows on each side, zero-filled past the global image edges).

Per-core pipeline (all f32):
  1. channel sum folded into the input DMA itself: channel 0 lands via a
     plain HWDGE DMA, channels 1/2 via SWDGE accum_op=add DMAs (CCE inline
     adds) -- no vector-engine work at all
  2. vertical 31-tap box sum via two banded 0/1-matrix fp32r matmuls per
     PSUM bank (bands are compile-time constant inputs)
  3. ACT copies PSUM -> two zero-prefixed SBUF chunk buffers (W halves)
  4. horizontal 31-tap box sum as two independent half-width
     tensor_tensor_scans (each chunk has its own 31-col zero prefix, so
     the first 30 lanes are warmup and are not stored)
  5. DMA result half-rows to DRAM
"""

import numpy as np

R = 15
TAP = 2 * R + 1          # 31
B, C, H, W = 4, 3, 2048, 2048
HALF = H // 2            # 1024 output rows per core
S_ROWS = HALF + 32       # 1056 input rows per core (16-row halo each side)
N_CORES = 8
P = 128                  # SBUF partitions
N_OUT_TILES = HALF // P  # 8
TAIL_ROWS = S_ROWS - N_OUT_TILES * P  # 32 valid rows in the 9th s-tile
MM_N = 512               # fp32 moving-operand max / one PSUM bank

# Horizontal W-half chunk geometry. Each chunk buffer is
# [31 zeros][t-columns][right zeros] and is scanned independently; the
# first 30 scan lanes are warmup and discarded.
#   chunk A: t cols [0, 1039)    -> out cols [0, 1024),     out j at s=j+15
#   chunk B: t cols [1009, 2048) -> out cols [1024, 2048),  out j at s=j-994
CA_N = 1039              # t cols 0..1038 (1024 + 15 right overlap)
CB_T0 = 1009             # chunk B t-col origin (1024 - 15)
CB_N = 1039              # t cols 1009..2047
XPA_W = TAP + CA_N       # 1070
XPB_W = TAP + CB_N + R   # 1085 (15 right zeros for the edge clamp)
SCAN_A = CA_N            # scan steps; store box_a[15 : 15+1024]
SCAN_B = CB_N + R        # 1054;      store box_b[30 : 30+1024]

_CACHE = {}


def _band_matrices():
    # out row i of a 128-row tile needs halo'd input rows r = i+1 .. i+31
    # (r is the row index within the [s_lo; s_hi] 256-row window).
    k = np.arange(P)[:, None]
    i = np.arange(P)[None, :]
    band_a = ((k >= i + 1) & (k <= i + TAP)).astype(np.float32)          # rows in s_lo
    band_b = ((k + P >= i + 1) & (k + P <= i + TAP)).astype(np.float32)  # rows in s_hi
    return band_a, band_b


def _build_kernel(tc, nc, out, xs, band_a_d, band_b_d, mybir, bass):
    from contextlib import ExitStack

    f32 = mybir.dt.float32
    f32r = mybir.dt.float32r
    add = mybir.AluOpType.add
    sub = mybir.AluOpType.subtract

    with ExitStack() as ctx:
        const_pool = ctx.enter_context(tc.tile_pool(name="const", bufs=1))
        s_pool = ctx.enter_context(tc.tile_pool(name="s", bufs=6))
        box_pool = ctx.enter_context(tc.tile_pool(name="box", bufs=4))
        psum_pool = ctx.enter_context(
            tc.tile_pool(name="psum", bufs=8, space=bass.MemorySpace.PSUM)
        )

        band_a = const_pool.tile([P, P], f32r)
        band_b = const_pool.tile([P, P], f32r)
        nc.sync.dma_start(band_a[:], band_a_d)
        nc.sync.dma_start(band_b[:], band_b_d)

        # Static double-buffered chunk tiles; zero pads are written once.
        # bf16: the 2e-2 tolerance leaves plenty of headroom, the scan keeps
        # fp32 internal state (and the +/- telescoping cancels exactly), and
        # the result stores halve. The host widens the output back to f32.
        bf16 = mybir.dt.bfloat16
        xp_a = [const_pool.tile([P, XPA_W], bf16, name=f"xp_a{i}") for i in (0, 1)]
        xp_b = [const_pool.tile([P, XPB_W], bf16, name=f"xp_b{i}") for i in (0, 1)]
        for xa, xb in zip(xp_a, xp_b):
            nc.gpsimd.memset(xa[:, 0:TAP], 0.0)
            nc.gpsimd.memset(xb[:, 0:TAP], 0.0)
            nc.gpsimd.memset(xb[:, TAP + CB_N : XPB_W], 0.0)

        xc_pool = ctx.enter_context(tc.tile_pool(name="xc", bufs=4))

        # Input loads live ONLY on the sync HWDGE queue and stores ONLY on
        # scalar: a store that waits on its scan must never head-of-line
        # block input prefetch, or the SDMA engines run dry mid-kernel.
        # Channel adds run on DVE/GpSimd; the 32-row tail tile uses SWDGE
        # accum DMAs instead so the final dependency chain is shorter.
        ACCUM = {7, 8}
        GP_ADD = {2, 4}

        def make_s(u):
            # Channel-summed input rows. Rows past the 32-row tail are
            # stale pool data -- finite, and multiplied by zero band weights.
            rows = P if u < N_OUT_TILES else TAIL_ROWS
            s = s_pool.tile([P, W], f32r)
            if u in ACCUM:
                # c0 plain HWDGE write, then c1/c2 accumulate in the DMA
                # datapath (CCE inline adds) -- no engine work.
                sd = s[:rows, :]
                nc.sync.dma_start(sd, xs[0, P * u : P * u + rows, :])
                nc.gpsimd.dma_start(sd, xs[1, P * u : P * u + rows, :], accum_op=add)
                nc.gpsimd.dma_start(sd, xs[2, P * u : P * u + rows, :], accum_op=add)
                return s
            if u == 3:
                # probe: SWDGE cast-DMA (f32 -> bf16) throughput
                xb3 = xc_pool.tile([P, C, W], bf16, name="xb3", bufs=1)
                nc.gpsimd.dma_start(
                    xb3[:rows],
                    xs[:, P * u : P * u + rows, :]
                    .bitcast(f32)
                    .rearrange("c p n -> p c n"),
                )
                nc.vector.tensor_add(s[:rows, :], xb3[:rows, 0, :], xb3[:rows, 1, :])
                nc.vector.tensor_add(s[:rows, :], s[:rows, :], xb3[:rows, 2, :])
                return s
            # c0 straight into s, c1/c2 batched into a side tile, adds on an
            # engine (f32r output satisfies the fp32r-matmul producer check)
            nc.sync.dma_start(s[:rows, :], xs[0, P * u : P * u + rows, :])
            xc = xc_pool.tile([P, 2, W], f32)
            nc.sync.dma_start(
                xc[:rows],
                xs[1:3, P * u : P * u + rows, :]
                .bitcast(f32)
                .rearrange("c p n -> p c n"),
            )
            eng = nc.gpsimd if u in GP_ADD else nc.vector
            eng.tensor_add(s[:rows, :], s[:rows, :].bitcast(f32), xc[:rows, 0, :])
            eng.tensor_add(s[:rows, :], s[:rows, :], xc[:rows, 1, :])
            return s

        # Stores are issued two iterations late so a store waiting on its
        # scan never head-of-line blocks the next tile's PSUM copies in the
        # scalar engine queue.
        pending_stores = []

        def flush_stores(upto):
            while pending_stores and pending_stores[0][0] <= upto:
                _, ba, bb, t0 = pending_stores.pop(0)
                nc.scalar.dma_start(
                    out[P * t0 : P * (t0 + 1), 0:HALF], ba[:, R : R + HALF]
                )
                nc.scalar.dma_start(
                    out[P * t0 : P * (t0 + 1), HALF:W], bb[:, 2 * R : 2 * R + HALF]
                )

        s_tiles = {0: make_s(0)}
        for t in range(N_OUT_TILES):
            s_tiles[t + 1] = make_s(t + 1)
            flush_stores(t - 2)
            s_lo, s_hi = s_tiles.pop(t), s_tiles[t + 1]

            # all band_a matmuls, then all band_b: minimizes PE weight reloads
            psums = []
            for nb in range(W // MM_N):
                ps = psum_pool.tile([P, MM_N], f32)
                nc.tensor.matmul(
                    ps[:], band_a[:], s_lo[:, MM_N * nb : MM_N * (nb + 1)],
                    start=True, stop=False,
                )
                psums.append(ps)
            for nb in range(W // MM_N):
                nc.tensor.matmul(
                    psums[nb][:], band_b[:], s_hi[:, MM_N * nb : MM_N * (nb + 1)],
                    start=False, stop=True,
                )

            xa, xb = xp_a[t % 2], xp_b[t % 2]
            # chunk A: t[0:1039] = ps0 | ps1 | ps2[:,0:15]
            nc.scalar.copy(xa[:, TAP : TAP + MM_N], psums[0][:])
            nc.scalar.copy(xa[:, TAP + MM_N : TAP + 2 * MM_N], psums[1][:])
            nc.scalar.copy(xa[:, TAP + 2 * MM_N : TAP + CA_N], psums[2][:, 0 : CA_N - 2 * MM_N])
            # chunk B: t[1009:2048] = ps1[:,497:512] | ps2 | ps3
            nb0 = 2 * MM_N - CB_T0  # 15 cols from ps1
            nc.scalar.copy(xb[:, TAP : TAP + nb0], psums[1][:, CB_T0 - MM_N : MM_N])
            nc.scalar.copy(xb[:, TAP + nb0 : TAP + nb0 + MM_N], psums[2][:])
            nc.scalar.copy(xb[:, TAP + nb0 + MM_N : TAP + CB_N], psums[3][:])

            box_a = box_pool.tile([P, SCAN_A], bf16)
            box_b = box_pool.tile([P, SCAN_B], bf16)
            nc.vector.tensor_tensor_scan(
                box_a[:], xa[:, TAP : TAP + SCAN_A], xa[:, 0:SCAN_A], 0.0, add, sub
            )
            nc.vector.tensor_tensor_scan(
                box_b[:], xb[:, TAP : TAP + SCAN_B], xb[:, 0:SCAN_B], 0.0, add, sub
            )
            pending_stores.append((t, box_a, box_b, t))
        flush_stores(N_OUT_TILES)


def _get_nc():
    if "nc" in _CACHE:
        return _CACHE["nc"]
    import concourse.bass as bass
    import concourse.tile as tile
    from concourse import bacc, mybir

    nc = bacc.Bacc(
        "TRN2", target_bir_lowering=False, debug=False, num_devices=N_CORES
    )
    # float32r so the DMA-loaded moving operand satisfies the fp32r-matmul
    # producer check; host feeds raw f32 bits (worst case the PE truncates
    # low mantissa bits -- far below the needed precision).
    xs = nc.dram_tensor("xs", [C, S_ROWS, W], mybir.dt.float32r, kind="ExternalInput")
    ba = nc.dram_tensor("band_a", [P, P], mybir.dt.float32r, kind="ExternalInput")
    bb = nc.dram_tensor("band_b", [P, P], mybir.dt.float32r, kind="ExternalInput")
    out = nc.dram_tensor("out", [HALF, W], mybir.dt.bfloat16, kind="ExternalOutput")

    with tile.TileContext(nc) as tc:
        _build_kernel(tc, nc, out.ap(), xs.ap(), ba.ap(), bb.ap(), mybir, bass)
    nc.compile()
    _CACHE["nc"] = nc
    return nc


def _in_maps(x):
    band_a, band_b = _band_matrices()
    maps = []
    for k in range(N_CORES):
        b, half = divmod(k, 2)
        h0 = half * HALF
        lo = h0 - 16  # global row of xs row 0
        g0, g1 = max(lo, 0), min(h0 + HALF + 16, H)
        xs = np.zeros((C, S_ROWS, W), np.float32)
        xs[:, g0 - lo : g1 - lo, :] = x[b, :, g0:g1, :]
        maps.append({"xs": xs, "band_a": band_a, "band_b": band_b})
    return maps


def _run(x, trace=False, tmpdir=None):
    from concourse.bass_utils import run_bass_kernel_spmd

    nc = _get_nc()
    res = run_bass_kernel_spmd(
        nc, _in_maps(x), list(range(N_CORES)), trace=trace, tmpdir=tmpdir
    )
    out = np.empty((B, 1, H, W), np.float32)
    for k in range(N_CORES):
        b, half = divmod(k, 2)
        out[b, 0, half * HALF : (half + 1) * HALF, :] = np.asarray(
            res.results[k]["out"]
        ).astype(np.float32)
    return out, res


def kernel(x: np.ndarray) -> np.ndarray:
    x = np.ascontiguousarray(x, dtype=np.float32)
    assert x.shape == (B, C, H, W)
    return _run(x)[0]


# revision 24
# speedup vs baseline: 1.0679x; 1.0679x over previous
"""BoxFilter kernel for Trainium2 (8 NeuronCores).

Computes out[b,0,i,j] = sum_{c} sum_{|di|<=15} sum_{|dj|<=15} x[b,c,i+di,j+dj]
(edge-clamped 31x31 box filter over the channel-summed image), matching the
reference cumsum + shifted-diff formulation exactly (separable box sums).

Sharding: data-parallel over (batch, H-half) -> 8 shards, no cross-core
communication. Each core receives a host-padded [3, 1056, 2048] slab
(16 halo rows on each side, zero-filled past the global image edges).

Per-core pipeline (all f32):
  1. channel sum folded into the input DMA itself: channel 0 lands via a
     plain HWDGE DMA, channels 1/2 via SWDGE accum_op=add DMAs (CCE inline
     adds) -- no vector-engine work at all
  2. vertical 31-tap box sum via two banded 0/1-matrix fp32r matmuls per
     PSUM bank (bands are compile-time constant inputs)
  3. ACT copies PSUM -> two zero-prefixed SBUF chunk buffers (W halves)
  4. horizontal 31-tap box sum as two independent half-width
     tensor_tensor_scans (each chunk has its own 31-col zero prefix, so
     the first 30 lanes are warmup and are not stored)
  5. DMA result half-rows to DRAM
"""

import numpy as np

R = 15
TAP = 2 * R + 1          # 31
B, C, H, W = 4, 3, 2048, 2048
HALF = H // 2            # 1024 output rows per core
S_ROWS = HALF + 32       # 1056 input rows per core (16-row halo each side)
N_CORES = 8
P = 128                  # SBUF partitions
N_OUT_TILES = HALF // P  # 8
TAIL_ROWS = S_ROWS - N_OUT_TILES * P  # 32 valid rows in the 9th s-tile
MM_N = 512               # fp32 moving-operand max / one PSUM bank

# Horizontal W-half chunk geometry. Each chunk buffer is
# [31 zeros][t-columns][right zeros] and is scanned independently; the
# first 30 scan lanes are warmup and discarded.
#   chunk A: t cols [0, 1039)    -> out cols [0, 1024),     out j at s=j+15
#   chunk B: t cols [1009, 2048) -> out cols [1024, 2048),  out j at s=j-994
CA_N = 1039              # t cols 0..1038 (1024 + 15 right overlap)
CB_T0 = 1009             # chunk B t-col origin (1024 - 15)
CB_N = 1039              # t cols 1009..2047
XPA_W = TAP + CA_N       # 1070
XPB_W = TAP + CB_N + R   # 1085 (15 right zeros for the edge clamp)
SCAN_A = CA_N            # scan steps; store box_a[15 : 15+1024]
SCAN_B = CB_N + R        # 1054;      store box_b[30 : 30+1024]

_CACHE = {}


def _band_matrices():
    # out row i of a 128-row tile needs halo'd input rows r = i+1 .. i+31
    # (r is the row index within the [s_lo; s_hi] 256-row window).
    k = np.arange(P)[:, None]
    i = np.arange(P)[None, :]
    band_a = ((k >= i + 1) & (k <= i + TAP)).astype(np.float32)          # rows in s_lo
    band_b = ((k + P >= i + 1) & (k + P <= i + TAP)).astype(np.float32)  # rows in s_hi
    return band_a, band_b


def _build_kernel(tc, nc, out, xs, band_a_d, band_b_d, mybir, bass):
    from contextlib import ExitStack

    f32 = mybir.dt.float32
    f32r = mybir.dt.float32r
    add = mybir.AluOpType.add
    sub = mybir.AluOpType.subtract

    with ExitStack() as ctx:
        const_pool = ctx.enter_context(tc.tile_pool(name="const", bufs=1))
        s_pool = ctx.enter_context(tc.tile_pool(name="s", bufs=6))
        box_pool = ctx.enter_context(tc.tile_pool(name="box", bufs=4))
        psum_pool = ctx.enter_context(
            tc.tile_pool(name="psum", bufs=8, space=bass.MemorySpace.PSUM)
        )

        band_a = const_pool.tile([P, P], f32r)
        band_b = const_pool.tile([P, P], f32r)
        nc.sync.dma_start(band_a[:], band_a_d)
        nc.sync.dma_start(band_b[:], band_b_d)

        # Static double-buffered chunk tiles; zero pads are written once.
        # bf16: the 2e-2 tolerance leaves plenty of headroom, the scan keeps
        # fp32 internal state (and the +/- telescoping cancels exactly), and
        # the result stores halve. The host widens the output back to f32.
        bf16 = mybir.dt.bfloat16
        xp_a = [const_pool.tile([P, XPA_W], bf16, name=f"xp_a{i}") for i in (0, 1)]
        xp_b = [const_pool.tile([P, XPB_W], bf16, name=f"xp_b{i}") for i in (0, 1)]
        for xa, xb in zip(xp_a, xp_b):
            nc.gpsimd.memset(xa[:, 0:TAP], 0.0)
            nc.gpsimd.memset(xb[:, 0:TAP], 0.0)
            nc.gpsimd.memset(xb[:, TAP + CB_N : XPB_W], 0.0)

        xc_pool = ctx.enter_context(tc.tile_pool(name="xc", bufs=4))

        # Input loads live ONLY on the sync HWDGE queue and stores ONLY on
        # scalar: a store that waits on its scan must never head-of-line
        # block input prefetch, or the SDMA engines run dry mid-kernel.
        # Channel adds run on DVE/GpSimd; the 32-row tail tile uses SWDGE
        # accum DMAs instead so the final dependency chain is shorter.
        ACCUM = {6, 7, 8}
        GP_ADD = {2, 4}

        def make_s(u):
            # Channel-summed input rows. Rows past the 32-row tail are
            # stale pool data -- finite, and multiplied by zero band weights.
            rows = P if u < N_OUT_TILES else TAIL_ROWS
            s = s_pool.tile([P, W], f32r)
            if u in ACCUM:
                # c0 plain HWDGE write, then c1/c2 accumulate in the DMA
                # datapath (CCE inline adds) -- no engine work.
                sd = s[:rows, :]
                nc.sync.dma_start(sd, xs[0, P * u : P * u + rows, :])
                nc.gpsimd.dma_start(sd, xs[1, P * u : P * u + rows, :], accum_op=add)
                nc.gpsimd.dma_start(sd, xs[2, P * u : P * u + rows, :], accum_op=add)
                return s
            # c0 straight into s, c1/c2 batched into a side tile, adds on an
            # engine (f32r output satisfies the fp32r-matmul producer check)
            nc.sync.dma_start(s[:rows, :], xs[0, P * u : P * u + rows, :])
            xc = xc_pool.tile([P, 2, W], f32)
            nc.sync.dma_start(
                xc[:rows],
                xs[1:3, P * u : P * u + rows, :]
                .bitcast(f32)
                .rearrange("c p n -> p c n"),
            )
            eng = nc.gpsimd if u in GP_ADD else nc.vector
            eng.tensor_add(s[:rows, :], s[:rows, :].bitcast(f32), xc[:rows, 0, :])
            eng.tensor_add(s[:rows, :], s[:rows, :], xc[:rows, 1, :])
            return s

        # Stores are issued two iterations late so a store waiting on its
        # scan never head-of-line blocks the next tile's PSUM copies in the
        # scalar engine queue.
        pending_stores = []

        def flush_stores(upto):
            while pending_stores and pending_stores[0][0] <= upto:
                _, ba, bb, t0 = pending_stores.pop(0)
                nc.scalar.dma_start(
                    out[P * t0 : P * (t0 + 1), 0:HALF], ba[:, R : R + HALF]
                )
                nc.scalar.dma_start(
                    out[P * t0 : P * (t0 + 1), HALF:W], bb[:, 2 * R : 2 * R + HALF]
                )

        s_tiles = {0: make_s(0)}
        for t in range(N_OUT_TILES):
            s_tiles[t + 1] = make_s(t + 1)
            flush_stores(t - 2)
            s_lo, s_hi = s_tiles.pop(t), s_tiles[t + 1]

            # all band_a matmuls, then all band_b: minimizes PE weight reloads
            psums = []
            for nb in range(W // MM_N):
                ps = psum_pool.tile([P, MM_N], f32)
                nc.tensor.matmul(
                    ps[:], band_a[:], s_lo[:, MM_N * nb : MM_N * (nb + 1)],
                    start=True, stop=False,
                )
                psums.append(ps)
            for nb in range(W // MM_N):
                nc.tensor.matmul(
                    psums[nb][:], band_b[:], s_hi[:, MM_N * nb : MM_N * (nb + 1)],
                    start=False, stop=True,
                )

            xa, xb = xp_a[t % 2], xp_b[t % 2]
            # chunk A: t[0:1039] = ps0 | ps1 | ps2[:,0:15]
            nc.scalar.copy(xa[:, TAP : TAP + MM_N], psums[0][:])
            nc.scalar.copy(xa[:, TAP + MM_N : TAP + 2 * MM_N], psums[1][:])
            nc.scalar.copy(xa[:, TAP + 2 * MM_N : TAP + CA_N], psums[2][:, 0 : CA_N - 2 * MM_N])
            # chunk B: t[1009:2048] = ps1[:,497:512] | ps2 | ps3
            nb0 = 2 * MM_N - CB_T0  # 15 cols from ps1
            nc.scalar.copy(xb[:, TAP : TAP + nb0], psums[1][:, CB_T0 - MM_N : MM_N])
            nc.scalar.copy(xb[:, TAP + nb0 : TAP + nb0 + MM_N], psums[2][:])
            nc.scalar.copy(xb[:, TAP + nb0 + MM_N : TAP + CB_N], psums[3][:])

            box_a = box_pool.tile([P, SCAN_A], bf16)
            box_b = box_pool.tile([P, SCAN_B], bf16)
            nc.vector.tensor_tensor_scan(
                box_a[:], xa[:, TAP : TAP + SCAN_A], xa[:, 0:SCAN_A], 0.0, add, sub
            )
            nc.vector.tensor_tensor_scan(
                box_b[:], xb[:, TAP : TAP + SCAN_B], xb[:, 0:SCAN_B], 0.0, add, sub
            )
            pending_stores.append((t, box_a, box_b, t))
        flush_stores(N_OUT_TILES)


def _get_nc():
    if "nc" in _CACHE:
        return _CACHE["nc"]
    import concourse.bass as bass
    import concourse.tile as tile
    from concourse import bacc, mybir

    nc = bacc.Bacc(
        "TRN2", target_bir_lowering=False, debug=False, num_devices=N_CORES
    )
    # float32r so the DMA-loaded moving operand satisfies the fp32r-matmul
    # producer check; host feeds raw f32 bits (worst case the PE truncates
    # low mantissa bits -- far below the needed precision).
    xs = nc.dram_tensor("xs", [C, S_ROWS, W], mybir.dt.float32r, kind="ExternalInput")
    ba = nc.dram_tensor("band_a", [P, P], mybir.dt.float32r, kind="ExternalInput")
    bb = nc.dram_tensor("band_b", [P, P], mybir.dt.float32r, kind="ExternalInput")
    out = nc.dram_tensor("out", [HALF, W], mybir.dt.bfloat16, kind="ExternalOutput")

    with tile.TileContext(nc) as tc:
        _build_kernel(tc, nc, out.ap(), xs.ap(), ba.ap(), bb.ap(), mybir, bass)
    nc.compile()
    _CACHE["nc"] = nc
    return nc


def _in_maps(x):
    band_a, band_b = _band_matrices()
    maps = []
    for k in range(N_CORES):
        b, half = divmod(k, 2)
        h0 = half * HALF
        lo = h0 - 16  # global row of xs row 0
        g0, g1 = max(lo, 0), min(h0 + HALF + 16, H)
        xs = np.zeros((C, S_ROWS, W), np.float32)
        xs[:, g0 - lo : g1 - lo, :] = x[b, :, g0:g1, :]
        maps.append({"xs": xs, "band_a": band_a, "band_b": band_b})
    return maps


def _run(x, trace=False, tmpdir=None):
    from concourse.bass_utils import run_bass_kernel_spmd

    nc = _get_nc()
    res = run_bass_kernel_spmd(
        nc, _in_maps(x), list(range(N_CORES)), trace=trace, tmpdir=tmpdir
    )
    out = np.empty((B, 1, H, W), np.float32)
    for k in range(N_CORES):
        b, half = divmod(k, 2)
        out[b, 0, half * HALF : (half + 1) * HALF, :] = np.asarray(
            res.results[k]["out"]
        ).astype(np.float32)
    return out, res


def kernel(x: np.ndarray) -> np.ndarray:
    x = np.ascontiguousarray(x, dtype=np.float32)
    assert x.shape == (B, C, H, W)
    return _run(x)[0]
